# revision 1
# baseline (speedup 1.0000x reference)
"""Multi-head self-attention Trainium2 kernel (Bass/Tile), batch-parallel
over 8 NeuronCores. All-bf16 matmul operands (HW ~371us, rel err 4.3e-3).

Problem (hardcoded): B=8, L=1024, D=1024, H=16, hd=64, f32 in/out.
  qkv = x @ w_qkv + b_qkv ; per-head scores = q k^T / 8 ; mask ; softmax ;
  out = (P v) heads-merged @ w_out + b_out.

Sharding: one batch element per core (data parallel); full weights on every
core. No collectives.

Per-core dataflow (all matmuls bf16 operands, f32 PSUM accum):
  - phase 1a: qkT[m][128, L] = w_qk^T x^T (dim-major q,k; 2 heads per tile),
    ScalarE Identity evac with per-partition bias -> bf16.
  - phase 1b: v' token-major [128 tokens, 1040] = x w_v' where w_v' has a
    zero column appended per head; evac via Pool-engine add of a
    host-broadcast bias tile that puts 1.0 in the per-head 65th column
    (so PV's stationary carries a ones column -> softmax denominator for
    free).
  - scores: stationary k slice [64, 128] at partition offset ro, moving q
    slice [64, 512] same offset (PE tile_position handles K=64; no
    zero-padded copies). Exp via ScalarE with scale=1/8 and per-partition
    mask bias -> E bf16 (no max-subtraction; inputs bounded).
  - PV: po[65, L] += v'[c][:, h*65:(h+1)*65]^T E_c ; row 64 = denominator.
  - normalize off the PE: DVE reciprocal_approx_fast on the denom row,
    Pool partition_broadcast to 64 rows, one DVE multiply -> ot bf16.
  - phase 3: Y[lq] = ot^T @ w_out + b_out per Lq-tile, f32 out, straight
    to DRAM.
"""

import sys

import numpy as np

try:
    import concourse.bass as bass  # noqa: F401
except Exception:  # pragma: no cover - defensive path setup
    for p in ("/opt/trn_rl_repo", "/opt/pypackages"):
        if p not in sys.path:
            sys.path.insert(0, p)
    import concourse.bass as bass  # noqa: F401

from contextlib import ExitStack

import concourse.tile as tile
from concourse import bacc, mybir
from concourse.bass_utils import run_bass_kernel_spmd

F32 = mybir.dt.float32
BF16 = mybir.dt.bfloat16

B, L, D = 8, 1024, 1024
H, HD = 16, 64
N_CORES = 8
PART = 128
NK = D // PART  # 8 x-dim contraction chunks
NMQK = 2 * D // PART  # 16 qk output tiles
NLQ = L // PART  # 8 query tiles
NLK = L // PART  # 8 key tiles
NPAIR = H // 2
VW = H * (HD + 1)  # 1040: per-head 64 dims + ones col
VG = 4  # v' psum column groups (each fits a 2KB bank)
VGW = VW // VG  # 260
EXP = mybir.ActivationFunctionType.Exp
IDENT = mybir.ActivationFunctionType.Identity


def build_nc(debug=False):
    nc = bacc.Bacc("TRN2", target_bir_lowering=False, debug=False)

    xT = nc.dram_tensor("xT", (D, L), BF16, kind="ExternalInput").ap()
    # wqk_blk[m, p, k, c] = w_qkv[k*128 + p, m*128 + c], m < 16 (q then k)
    wqk_blk = nc.dram_tensor(
        "wqk_blk", (NMQK, PART, NK, PART), BF16, kind="ExternalInput"
    ).ap()
    # bqk[p, m] = b_qkv[m*128 + p]
    bqk = nc.dram_tensor("bqk", (PART, NMQK), F32, kind="ExternalInput").ap()
    # wv_blk[k, p, n] = w_v'[k*128 + p, n]  (w_v' 1040-wide, zero ones-cols)
    wv_blk = nc.dram_tensor("wv_blk", (NK, PART, VW), BF16, kind="ExternalInput").ap()
    # vbias[p, n]: b_v at dim cols, 1.0 at ones cols (broadcast over p)
    vbias = nc.dram_tensor("vbias", (PART, VW), F32, kind="ExternalInput").ap()
    # maskb[p, c] = 0 / -10000 for key c*128+p
    maskb = nc.dram_tensor("maskb", (PART, NLK), F32, kind="ExternalInput").ap()
    # wo_blk[k, p, n] = w_out[k*128 + p, n]
    wo_blk = nc.dram_tensor("wo_blk", (NK, PART, D), BF16, kind="ExternalInput").ap()
    bout = nc.dram_tensor("bout", (PART, D), F32, kind="ExternalInput").ap()
    Y = nc.dram_tensor("Y", (L, D), F32, kind="ExternalOutput").ap()
    dbg = {}
    if debug:
        for nm, shp, dt in [
            ("dbg_q", (PART, L), BF16), ("dbg_k", (PART, L), BF16),
            ("dbg_vv", (PART, VW), BF16), ("dbg_e", (PART, L), BF16),
            ("dbg_rcp", (1, L), F32), ("dbg_rtb", (HD, L), F32),
            ("dbg_po", (HD + 1, L), F32), ("dbg_ot", (PART, L), BF16),
        ]:
            dbg[nm] = nc.dram_tensor(nm, shp, dt, kind="ExternalOutput").ap()

    with tile.TileContext(nc) as tc, ExitStack() as ctx:
        singles = ctx.enter_context(tc.tile_pool(name="singles", bufs=1))

        mb_sb = singles.tile([PART, NLK], F32)
        nc.sync.dma_start(mb_sb[:], maskb[:, :])
        bqk_sb = singles.tile([PART, NMQK], F32)
        nc.sync.dma_start(bqk_sb[:], bqk[:, :])
        vbias_sb = singles.tile([PART, VW], F32)

        # ---- persistent tiles ----
        xt_pool = ctx.enter_context(tc.tile_pool(name="xt", bufs=1))
        xt = [xt_pool.tile([PART, L], BF16, tag=f"xt{k}", name=f"xt{k}") for k in range(NK)]
        wv_pool = ctx.enter_context(tc.tile_pool(name="wv", bufs=1))
        wv_sb = [wv_pool.tile([PART, VW], BF16, tag=f"wv{k}", name=f"wv{k}") for k in range(NK)]
        qk_pool = ctx.enter_context(tc.tile_pool(name="qk", bufs=1))
        qt = [qk_pool.tile([PART, L], BF16, tag=f"q{j}", name=f"q{j}") for j in range(NPAIR)]
        kt = [qk_pool.tile([PART, L], BF16, tag=f"k{j}", name=f"k{j}") for j in range(NPAIR)]
        vv_pool = ctx.enter_context(tc.tile_pool(name="vv", bufs=1))
        vv = [vv_pool.tile([PART, VW], BF16, tag=f"vv{c}", name=f"vv{c}") for c in range(NLK)]
        ot_pool = ctx.enter_context(tc.tile_pool(name="ot", bufs=1))
        ot_fin = [ot_pool.tile([PART, L], BF16, tag=f"ot{j}", name=f"ot{j}") for j in range(NPAIR)]
        wo_pool = ctx.enter_context(tc.tile_pool(name="wo", bufs=1))

        # qk weight stream + 1a helper
        wqk_pool = ctx.enter_context(tc.tile_pool(name="wqkp", bufs=3))
        mt_seq = [j + 8 * s for j in range(NPAIR) for s in range(2)]  # q0,k0,q1,k1,...
        mt_loaded = {}

        def load_mtile(i):
            if i >= NMQK:
                return
            m = mt_seq[i]
            wt = wqk_pool.tile([PART, NK * PART], BF16, tag="wqk")
            nc.sync.dma_start(wt[:], wqk_blk[m].rearrange("p k c -> p (k c)"))
            mt_loaded[i] = wt

        def compute_mtile(i, p1_pool):
            m = mt_seq[i]
            wt = mt_loaded.pop(i)
            dst = qt[m] if m < NPAIR else kt[m - NPAIR]
            ph = p1_pool.tile([PART, L], F32, tag="p1")
            for k in range(NK):
                for nh in range(2):
                    nc.tensor.matmul(
                        ph[:, nh * 512 : (nh + 1) * 512],
                        wt[:, k * PART : (k + 1) * PART],
                        xt[k][:, nh * 512 : (nh + 1) * 512],
                        start=(k == 0),
                        stop=(k == NK - 1),
                    )
            nc.scalar.activation(
                dst[:], ph[:], IDENT, bias=bqk_sb[:, m : m + 1], scale=1.0
            )

        # attention helpers
        e_pool = ctx.enter_context(tc.tile_pool(name="epool", bufs=2 * NLK))
        rcp_pool = ctx.enter_context(tc.tile_pool(name="rcp", bufs=2))
        rtb_pool = ctx.enter_context(tc.tile_pool(name="rtb", bufs=2))
        ets = {}

        def emit_score_chunk(h, c, st_pool):
            j, ro = h // 2, (h % 2) * HD
            st = st_pool.tile([PART, L], F32, tag="st")
            for nh in range(2):
                nc.tensor.matmul(
                    st[:, nh * 512 : (nh + 1) * 512],
                    kt[j][ro : ro + HD, c * PART : (c + 1) * PART],
                    qt[j][ro : ro + HD, nh * 512 : (nh + 1) * 512],
                    start=True,
                    stop=True,
                )
            et = e_pool.tile([PART, L], BF16, tag="e", name="et")
            nc.scalar.activation(
                et[:], st[:], EXP, bias=mb_sb[:, c : c + 1], scale=1.0 / 8.0
            )
            if debug and h == 0 and c == 0:
                nc.sync.dma_start(dbg["dbg_e"][:, :], et[:])
            ets[h].append(et)

        def emit_pv_chunk(h, c, po_t):
            for nh in range(2):
                nc.tensor.matmul(
                    po_t[0 : HD + 1, nh * 512 : (nh + 1) * 512],
                    vv[c][:, h * (HD + 1) : (h + 1) * (HD + 1)],
                    ets[h][c][:, nh * 512 : (nh + 1) * 512],
                    start=(c == 0),
                    stop=(c == NLK - 1),
                )

        def emit_finish(h, po_t):
            j, ro = h // 2, (h % 2) * HD
            den = rcp_pool.tile([1, L], F32, tag="den")
            nc.scalar.copy(den[:], po_t[HD : HD + 1, :])
            rcp = rcp_pool.tile([1, L], F32, tag="rcp")
            with nc.allow_low_precision(reason="softmax denom reciprocal"):
                nc.vector.reciprocal_approx_fast(rcp[:], den[:])
            rtb = rtb_pool.tile([HD, L], F32, tag="rtb")
            nc.gpsimd.partition_broadcast(rtb[:], rcp[:], channels=HD)
            nc.vector.tensor_mul(
                ot_fin[j][ro : ro + HD, :], po_t[0:HD, :], rtb[:]
            )
            if debug and h == 0:
                nc.sync.dma_start(dbg["dbg_rcp"][:, :], rcp[:])
                nc.sync.dma_start(dbg["dbg_rtb"][:, :], rtb[:])
                posc = rtb_pool.tile([HD + 1, L], F32, tag="podbg", name="podbg")
                nc.vector.tensor_copy(posc[:], po_t[0 : HD + 1, :])
                nc.sync.dma_start(dbg["dbg_po"][:, :], posc[:])

        # ============ phase 1 + bootstrap ============
        st_ctx = ExitStack()
        st_pool = st_ctx.enter_context(
            tc.tile_pool(name="stp", bufs=1, space="PSUM", side="right")
        )
        p1_ctx = ExitStack()
        p1_pool = p1_ctx.enter_context(
            tc.tile_pool(name="p1", bufs=1, space="PSUM", side="right")
        )

        # input DMAs in consumption order: first qk-weight tiles 0-1, then
        # xt (1a contracts k progressively), then the rest
        load_mtile(0)
        load_mtile(1)
        for k in range(NK):
            nc.sync.dma_start(xt[k][:, 0:512], xT[k * PART : (k + 1) * PART, 0:512])
            nc.sync.dma_start(xt[k][:, 512:L], xT[k * PART : (k + 1) * PART, 512:L])
        load_mtile(2)
        load_mtile(3)
        nc.sync.dma_start(vbias_sb[:], vbias[:, :])
        for k in range(NK):
            nc.sync.dma_start(wv_sb[k][:, 0:520], wv_blk[k][:, 0:520])
            nc.sync.dma_start(wv_sb[k][:, 520:VW], wv_blk[k][:, 520:VW])
        for i in range(4):
            compute_mtile(i, p1_pool)
        load_mtile(4)

        # 1b (v' token-major) interleaved with head-0 scores
        ets[0] = []
        with tc.tile_pool(name="vbp", bufs=1, space="PSUM") as vb_pool:
            for c in range(NLK):
                pvs = [
                    vb_pool.tile([PART, VGW], F32, tag=f"vb{g}", name=f"vb{g}")
                    for g in range(VG)
                ]
                for k in range(NK):
                    for g in range(VG):
                        nc.tensor.matmul(
                            pvs[g][:],
                            xt[k][:, c * PART : (c + 1) * PART],
                            wv_sb[k][:, g * VGW : (g + 1) * VGW],
                            start=(k == 0),
                            stop=(k == NK - 1),
                        )
                for g in range(VG):
                    nc.vector.tensor_add(
                        vv[c][:, g * VGW : (g + 1) * VGW],
                        pvs[g][:],
                        vbias_sb[:, g * VGW : (g + 1) * VGW],
                    )
                emit_score_chunk(0, c, st_pool)

        # wo / bout DMAs (needed only in phase 3)
        bout_sb = wo_pool.tile([PART, D], F32, tag="bout")
        nc.sync.dma_start(bout_sb[:], bout[:, :])
        wo_sb = []
        for k in range(NK):
            t = wo_pool.tile([PART, D], BF16, tag=f"wo{k}")
            nc.sync.dma_start(t[:], wo_blk[k])
            wo_sb.append(t)

        # ============ phase 2 main loop (+ remaining 1a) ============
        po_ctx = ExitStack()
        po_pool = po_ctx.enter_context(
            tc.tile_pool(name="pop", bufs=2, space="PSUM")
        )
        # process pair 6 (heads 12,13) last so phase 3 (k-order ends at 6)
        # can start while the final pair is still in flight
        hseq = [0, 1, 2, 3, 4, 5, 6, 7, 8, 9, 10, 11, 14, 15, 12, 13]
        K_ORDER = [0, 1, 2, 3, 4, 5, 7, 6]
        pf_ctx = ExitStack()
        pf_pool = None
        pf_partial = []  # (lq, pf_tile) with k=6 pending

        def emit_pf(lq, pf_t, ks):
            for k in ks:
                for nh in range(2):
                    nc.tensor.matmul(
                        pf_t[:, nh * 512 : (nh + 1) * 512],
                        ot_fin[k][:, lq * PART : (lq + 1) * PART],
                        wo_sb[k][:, nh * 512 : (nh + 1) * 512],
                        start=(k == K_ORDER[0]),
                        stop=(k == K_ORDER[-1]),
                    )

        for idx in range(H):
            h = hseq[idx]
            i = 5 + idx
            if i <= NMQK:
                load_mtile(i)
                compute_mtile(i - 1, p1_pool)
            if i == NMQK:
                p1_ctx.close()
            po_t = po_pool.tile([HD + 1, L], F32, tag="po")
            if idx + 1 < H:
                ets[hseq[idx + 1]] = []
            for c in range(NLK):
                if idx + 1 < H:
                    emit_score_chunk(hseq[idx + 1], c, st_pool)
                emit_pv_chunk(h, c, po_t)
            emit_finish(h, po_t)
            ets.pop(h)
            if idx == 14:
                # last score chunk (head 13) was emitted this iteration
                st_ctx.close()
                pf_pool = pf_ctx.enter_context(
                    tc.tile_pool(name="pf", bufs=1, space="PSUM", side="right")
                )
            if idx == 15 and pf_pool is not None:
                pf_t = pf_pool.tile([PART, D], F32, tag="pf")
                emit_pf(0, pf_t, K_ORDER[:-1])
                pf_partial.append((0, pf_t))
        po_ctx.close()
        if debug:
            nc.sync.dma_start(dbg["dbg_q"][:, :], qt[0][:])
            nc.sync.dma_start(dbg["dbg_k"][:, :], kt[0][:])
            nc.sync.dma_start(dbg["dbg_vv"][:, :], vv[0][:])
            nc.sync.dma_start(dbg["dbg_ot"][:, :], ot_fin[0][:])

        # ============ phase 3: output projection ============
        with tc.tile_pool(name="fsb", bufs=2) as f_pool:

            def evac_pf(lq, pf_t):
                fs = f_pool.tile([PART, D], F32, tag="fsb")
                for q in range(4):
                    ns = slice(q * 256, (q + 1) * 256)
                    nc.vector.tensor_add(fs[:, ns], pf_t[:, ns], bout_sb[:, ns])
                    nc.sync.dma_start(Y[lq * PART : (lq + 1) * PART, ns], fs[:, ns])

            for lq, pf_t in pf_partial:
                emit_pf(lq, pf_t, K_ORDER[-1:])
                evac_pf(lq, pf_t)
            pf_ctx.close()
            with tc.tile_pool(name="pf2", bufs=2, space="PSUM") as pf2_pool:
                for lq in range(len(pf_partial), NLQ):
                    pf_t = pf2_pool.tile([PART, D], F32, tag="pf")
                    emit_pf(lq, pf_t, K_ORDER)
                    evac_pf(lq, pf_t)

    nc.compile()
    return nc


_NC_CACHE = None


def _get_nc():
    global _NC_CACHE
    if _NC_CACHE is None:
        _NC_CACHE = build_nc()
    return _NC_CACHE


def make_in_maps(x, attn_mask, w_qkv, b_qkv, w_out, b_out):
    """Host-side sharding + layout prep -> per-core input maps."""
    import ml_dtypes

    bf16 = ml_dtypes.bfloat16
    x = np.asarray(x, dtype=np.float32)
    attn_mask = np.asarray(attn_mask)
    w_qkv = np.asarray(w_qkv, dtype=np.float32)
    b_qkv = np.asarray(b_qkv, dtype=np.float32)
    w_out = np.asarray(w_out, dtype=np.float32)
    b_out = np.asarray(b_out, dtype=np.float32)

    wqk = w_qkv[:, : 2 * D]  # (D, 2D)
    # wqk_blk[m, p, k, c] = wqk[k*128+p, m*128+c]
    wqk_blk = np.ascontiguousarray(
        wqk.reshape(NK, PART, NMQK, PART).transpose(2, 1, 0, 3).astype(bf16)
    )
    bqk_h = np.ascontiguousarray(b_qkv[: 2 * D].reshape(NMQK, PART).T)

    wv = w_qkv[:, 2 * D :]  # (D, D)
    wv_p = np.zeros((D, VW), dtype=np.float32)
    vbias_row = np.zeros((VW,), dtype=np.float32)
    for h in range(H):
        wv_p[:, h * (HD + 1) : h * (HD + 1) + HD] = wv[:, h * HD : (h + 1) * HD]
        vbias_row[h * (HD + 1) : h * (HD + 1) + HD] = b_qkv[
            2 * D + h * HD : 2 * D + (h + 1) * HD
        ]
        vbias_row[h * (HD + 1) + HD] = 1.0
    wv_blk_h = np.ascontiguousarray(wv_p.reshape(NK, PART, VW).astype(bf16))
    vbias_h = np.ascontiguousarray(np.broadcast_to(vbias_row, (PART, VW)))

    maskbias = np.where(attn_mask.astype(bool), 0.0, -10000.0).astype(np.float32)

    wo_blk_h = np.ascontiguousarray(w_out.reshape(NK, PART, D).astype(bf16))
    bout_h = np.ascontiguousarray(np.broadcast_to(b_out, (PART, D)))

    in_maps = []
    for b in range(B):
        in_maps.append(
            {
                "xT": np.ascontiguousarray(x[b].T.astype(bf16)),
                "wqk_blk": wqk_blk,
                "bqk": bqk_h,
                "wv_blk": wv_blk_h,
                "vbias": vbias_h,
                "maskb": np.ascontiguousarray(
                    maskbias[b].reshape(NLK, PART).T
                ),
                "wo_blk": wo_blk_h,
                "bout": bout_h,
            }
        )
    return in_maps


def kernel(x, attn_mask, w_qkv, b_qkv, w_out, b_out):
    in_maps = make_in_maps(x, attn_mask, w_qkv, b_qkv, w_out, b_out)
    nc = _get_nc()
    res = run_bass_kernel_spmd(nc, in_maps, core_ids=list(range(N_CORES)))
    return np.stack([res.results[b]["Y"] for b in range(B)], axis=0)


if __name__ == "__main__":
    rng = np.random.default_rng(0)
    inputs = {
        "x": rng.standard_normal((B, L, D), dtype=np.float32),
        "attn_mask": np.ones((B, L), dtype=bool),
        "w_qkv": ((rng.random((D, 3 * D), dtype=np.float32) - 0.5) / 16.0),
        "b_qkv": np.zeros((3 * D,), dtype=np.float32),
        "w_out": ((rng.random((D, D), dtype=np.float32) - 0.5) / 16.0),
        "b_out": np.zeros((D,), dtype=np.float32),
    }
    y = kernel(**inputs)
    print(y.shape, y.dtype)

